# revision 1
# baseline (speedup 1.0000x reference)
"""CRF forward (log-likelihood) kernel for Trainium2, 8 NeuronCores.

Strategy
--------
Data parallel over batch: each of the 8 cores gets B/8 = 512 sequences.

The denominator (log-partition, the compute-heavy part) is computed on
device in *exp space*: with A = exp(transitions), E_t = exp(emissions_t),

    alpha_{t} = (alpha_{t-1} @ A) * E_t

which maps the per-step transition contraction onto the TensorEngine as a
tiny matmul. To use more of the 128-partition contraction, 8 groups of 64
batch elements are packed block-diagonally: alpha lives as a [104, 64]
tile (104 = 8 groups x 13 tags, 64 batch columns per group), and A is a
[104, 104] block-diagonal matrix. Every 16 steps alpha is renormalized by
its per-sequence sum (computed with a block-diagonal ones matmul) and the
log of the normalizer is accumulated; this keeps fp32 from overflowing
(growth is ~e^3 per step). The end-transitions fold into the final sum's
weights. The numerator (score of the given tag path) is pure gathers and
is computed on the host.

The batch columns are split into NCH independent chains so the TensorE
matmul of one chain overlaps the VectorE emission-multiply of another
(the time recurrence itself is serial).
"""

import os
import numpy as np
from contextlib import ExitStack
from concurrent.futures import ThreadPoolExecutor

import concourse.bass as bass
import concourse.bacc as bacc
import concourse.mybir as mybir
import concourse.tile as tile
from concourse.bass_utils import run_bass_kernel_spmd

# Problem shape (hardcoded per contract)
B, T, K = 4096, 512, 13
NCORES = 8
BL = B // NCORES          # 512 sequences per core
G = 8                     # batch groups packed block-diagonally
BG = BL // G              # 64 batch columns per group
P = G * K                 # 104 partitions

S_RENORM = int(os.environ.get("CRF_S_RENORM", "128"))   # steps between renorms
CH = int(os.environ.get("CRF_CH", "32"))               # time steps per DMA/exp chunk
NCH = int(os.environ.get("CRF_NCH", "1"))              # batch column chains

_F32 = mybir.dt.float32
_EXP = mybir.ActivationFunctionType.Exp
_LN = mybir.ActivationFunctionType.Ln
C_SHIFT = 2.505  # mean per-step log-growth, folded into exp() bias

_cache = {}
LAST_RESULTS = None  # BassKernelResults of the most recent run (for test harness)


def _build_program():
    nc = bacc.Bacc()
    em_d = nc.dram_tensor("em_packed", [T - 1, P, BG], _F32, kind="ExternalInput")
    # All constants + alpha0 packed in one tensor -> one DMA -> the first
    # matmul carries a single semaphore wait (PE LDW has few wait slots).
    # cols: abd 0:104 | sum_w 104:112 | sum_w_end 112:120 | alpha0 120:184
    #       | bcast_w rows 0:8, cols 184:288
    cn_d = nc.dram_tensor("consts", [P, 288], _F32, kind="ExternalInput")
    out_d = nc.dram_tensor("denom_out", [G, BG], _F32, kind="ExternalOutput")

    W = BG // NCH
    n_chunks = (T - 1 + CH - 1) // CH

    with tile.TileContext(nc) as tc, ExitStack() as ctx:
        singles = ctx.enter_context(tc.tile_pool(name="singles", bufs=1))
        empool = ctx.enter_context(tc.tile_pool(name="em", bufs=2))
        epool = ctx.enter_context(tc.tile_pool(name="E", bufs=2))
        apool = ctx.enter_context(tc.tile_pool(name="alpha", bufs=4))
        sm = ctx.enter_context(tc.tile_pool(name="small", bufs=2))
        ps_a = ctx.enter_context(tc.tile_pool(name="ps_a", bufs=4, space="PSUM"))
        ps_b = ctx.enter_context(tc.tile_pool(name="ps_b", bufs=2, space="PSUM"))
        ps_s = ctx.enter_context(tc.tile_pool(name="ps_s", bufs=2, space="PSUM"))

        consts = singles.tile([P, 288], _F32)
        nc.sync.dma_start(consts[:], cn_d[:])
        abd = consts[:, 0:P]
        sw = consts[:, P:P + G]
        swe = consts[:, P + G:P + 2 * G]
        alpha_init = consts[:, 120:120 + BG]
        bw = consts[0:G, 184:184 + P]
        logacc = singles.tile([G, BG], _F32)
        nc.any.memset(logacc[:], 0.0)
        bias_t = singles.tile([P, 1], _F32)
        nc.any.memset(bias_t[:], -C_SHIFT)

        cur = [alpha_init[:, c * W:(c + 1) * W] for c in range(NCH)]

        reps = int(os.environ.get("CRF_REPS", "1"))  # >1: bench-only scaling
        for rep in range(reps):
         for t in range(1, T):
            j = (t - 1) // CH
            s = (t - 1) % CH
            if s == 0:
                steps = min(CH, (T - 1) - j * CH)
                emt = empool.tile([P, CH * BG], _F32, tag="em")
                src = em_d[j * CH: j * CH + steps, :, :].rearrange("s p b -> p s b")
                dst = emt[:, : steps * BG].rearrange("p (s b) -> p s b", s=steps)
                nc.sync.dma_start(dst, src)
                Et = epool.tile([P, CH * BG], _F32, tag="E")
                nc.scalar.activation(Et[:, : steps * BG], emt[:, : steps * BG], _EXP, bias=bias_t[:])
            nxt = []
            for c in range(NCH):
                pa = ps_a.tile([P, W], _F32, tag="psa")
                nc.tensor.matmul(pa[:], abd, cur[c], start=True, stop=True)
                na = apool.tile([P, W], _F32, tag="al")
                nc.vector.tensor_mul(
                    na[:], pa[:], Et[:, s * BG + c * W: s * BG + (c + 1) * W]
                )
                nxt.append(na[:])
            cur = nxt

            if (t % S_RENORM) == 0 and t < T - 1:
                sps, rcs = [], []
                for c in range(NCH):
                    sp = ps_s.tile([G, W], _F32, tag="pss")
                    nc.tensor.matmul(sp[:], sw, cur[c], start=True, stop=True)
                    sps.append(sp)
                for c in range(NCH):
                    rc = sm.tile([G, W], _F32, tag="rc")
                    nc.vector.reciprocal(rc[:], sps[c][:])
                    rcs.append(rc)
                    ln = sm.tile([G, W], _F32, tag="ln")
                    nc.scalar.activation(ln[:], sps[c][:], _LN)
                    nc.vector.tensor_add(
                        logacc[:, c * W:(c + 1) * W],
                        logacc[:, c * W:(c + 1) * W],
                        ln[:],
                    )
                rn_ps = []
                for c in range(NCH):
                    bp = ps_b.tile([P, W], _F32, tag="psb")
                    nc.tensor.matmul(bp[:], bw, rcs[c][:], start=True, stop=True)
                    rn_ps.append(bp)
                nxt = []
                for c in range(NCH):
                    rn = apool.tile([P, W], _F32, tag="al")
                    nc.vector.tensor_mul(rn[:], rn_ps[c][:], cur[c])
                    nxt.append(rn[:])
                cur = nxt

        outsb = singles.tile([G, BG], _F32)
        for c in range(NCH):
            sp = ps_s.tile([G, W], _F32, tag="pss")
            nc.tensor.matmul(sp[:], swe, cur[c], start=True, stop=True)
            ln = sm.tile([G, W], _F32, tag="ln")
            nc.scalar.activation(ln[:], sp[:], _LN)
            nc.vector.tensor_add(
                outsb[:, c * W:(c + 1) * W], logacc[:, c * W:(c + 1) * W], ln[:]
            )
        nc.sync.dma_start(out_d[:], outsb[:])
    nc.finalize()
    return nc


def _numerator(em, tags, mask, start, end, trans):
    tags = tags.astype(np.int64)
    maskf = mask.astype(np.float32)
    emit = np.take_along_axis(em, tags[..., None], axis=2)[..., 0]
    tr = trans[tags[:, :-1], tags[:, 1:]]
    num = start[tags[:, 0]] + emit[:, 0]
    num = num + np.sum((tr + emit[:, 1:]) * maskf[:, 1:], axis=1)
    seq_ends = mask.astype(np.int32).sum(1) - 1
    num = num + end[tags[np.arange(B), seq_ends]]
    return num


def _pack_core(em_core, start):
    # em_core: [BL, T, K] -> em_packed [T-1, P, BG]; alpha0 [P, BG]
    v = em_core.reshape(G, BG, T, K)
    packed = np.ascontiguousarray(v.transpose(2, 0, 3, 1)[1:]).reshape(T - 1, P, BG)
    a0 = np.exp(start[None, None, :, None] + v.transpose(2, 0, 3, 1)[0][None])
    a0 = np.ascontiguousarray(a0.astype(np.float32)).reshape(P, BG)
    return packed, a0


def kernel(emissions, tags, mask, start_transitions, end_transitions, transitions):
    global LAST_RESULTS
    em = np.ascontiguousarray(np.asarray(emissions, dtype=np.float32))
    tags = np.asarray(tags)
    mask = np.asarray(mask)
    start = np.asarray(start_transitions, dtype=np.float32)
    end = np.asarray(end_transitions, dtype=np.float32)
    trans = np.asarray(transitions, dtype=np.float32)

    num = _numerator(em, tags, mask, start, end, trans)

    # Pack per-core device inputs (threaded; numpy copies release the GIL)
    with ThreadPoolExecutor(NCORES) as ex:
        packs = list(
            ex.map(lambda c: _pack_core(em[c * BL:(c + 1) * BL], start), range(NCORES))
        )

    A = np.exp(trans).astype(np.float32)
    consts = np.zeros((P, 288), np.float32)
    for g in range(G):
        consts[g * K:(g + 1) * K, g * K:(g + 1) * K] = A          # abd
        consts[g * K:(g + 1) * K, P + g] = 1.0                     # sum_w
        consts[g * K:(g + 1) * K, P + G + g] = np.exp(end)         # sum_w_end
        consts[g, 184 + g * K:184 + (g + 1) * K] = 1.0             # bcast_w
    core_consts = []
    for c in range(NCORES):
        cc = consts.copy()
        cc[:, 120:120 + BG] = packs[c][1]                          # alpha0
        core_consts.append(cc)

    if "nc" not in _cache:
        _cache["nc"] = _build_program()
    nc = _cache["nc"]

    in_maps = [
        {"em_packed": packs[c][0], "consts": core_consts[c]}
        for c in range(NCORES)
    ]
    trace = bool(int(os.environ.get("CRF_TRACE", "0")))
    try:
        res = run_bass_kernel_spmd(
            nc, in_maps, core_ids=list(range(NCORES)), trace=trace
        )
    except ModuleNotFoundError:
        # NTFF profiling hook unavailable in this environment
        res = run_bass_kernel_spmd(
            nc, in_maps, core_ids=list(range(NCORES)), trace=False
        )
    LAST_RESULTS = res

    denom = np.concatenate(
        [res.results[c]["denom_out"].reshape(BL) for c in range(NCORES)]
    ) + np.float32((T - 1) * C_SHIFT)
    out = np.sum((num - denom).astype(np.float32), dtype=np.float64)
    return np.asarray(out, dtype=np.float32)

